# revision 2
# baseline (speedup 1.0000x reference)
"""DGMNet Trainium2 kernel, v2.

Changes vs v1 (fp32r baseline):
- All matmul operands and activation tiles in bf16 (PE rate unchanged at
  1 col/cycle, but halves DVE/ACT elementwise time, SBUF footprint, and
  weight-DMA bytes; rel err ~5e-3 vs the 2e-2 gate).
- The x-side projections (ux_z/g/r/h and S1) are computed ONCE per batch
  tile (5 gates x 8 m-tiles of K=16 matmuls) instead of re-issued as
  PSUM "start" matmuls inside every gate GEMM of every layer. That
  removes ~70 full-512-column K=16 PE streams per tile (~15 us/tile).
- Gate GEMMs accumulate on top of a PSUM preload of the ux tile (Pool
  engine tensor_copy bf16->f32), so no extra DVE adds and the ACT tanh
  evacuation needs no bias (biases are folded into ux host-side prep +
  phase-A ACT bias).
- Layer-0 R and G reuse the shared wgS1 GEMM via DVE adds issued
  immediately as each wgs m-tile lands, so layer-0 H never waits.
- (1-G) is precomputed once per tile as (G-1) in place; the combine is
  out = Z*S - (G-1)*H (3 DVE ops, bf16).

Data-parallel over batch: 65536 rows -> 8 cores x 8192.
"""

import sys

sys.path.insert(0, "/opt/trn_rl_repo")

import numpy as np

B_FULL = 65536
KI = 16
H = 1024
NCORES = 8
BC = B_FULL // NCORES  # per-core batch (8192)
NB = 512               # batch tile (one PSUM bank of fp32)
NM = H // 128          # feature tiles (8)
N_LAYERS = 4

MM_DT = "bfloat16"

_BUILD_CACHE = {}


def _build(bc, nb, mm_dt=MM_DT, repeat=1):
    import concourse.bacc as bacc
    import concourse.mybir as mybir
    import concourse.tile as tile

    f32 = mybir.dt.float32
    mdt = getattr(mybir.dt, mm_dt)
    Tanh = mybir.ActivationFunctionType.Tanh
    Ident = mybir.ActivationFunctionType.Identity
    sub = mybir.AluOpType.subtract

    nt = bc // nb

    nc = bacc.Bacc("TRN2", target_bir_lowering=False, debug=False,
                   num_devices=NCORES)

    xT_d = nc.dram_tensor("xT", [KI, bc], mdt, kind="ExternalInput").ap()
    wz_d = nc.dram_tensor("WzT", [H, H], mdt, kind="ExternalInput").ap()
    wg_d = nc.dram_tensor("WgT", [H, H], mdt, kind="ExternalInput").ap()
    u_d = nc.dram_tensor("U", [128, 5 * H], mdt, kind="ExternalInput").ap()
    bias_d = nc.dram_tensor("BIAS", [128, 48], f32, kind="ExternalInput").ap()
    ow_d = nc.dram_tensor("OW", [128, NM], mdt, kind="ExternalInput").ap()
    y_d = nc.dram_tensor("Y", [1, bc], f32, kind="ExternalOutput").ap()

    with tile.TileContext(nc) as tc:
        with (
            tc.tile_pool(name="const", bufs=1) as cpool,
            tc.tile_pool(name="xt", bufs=3) as xt_pool,
            tc.tile_pool(name="s", bufs=2) as s_pool,
            tc.tile_pool(name="ux", bufs=2) as ux_pool,
            tc.tile_pool(name="act", bufs=1) as act_pool,
            tc.tile_pool(name="ov", bufs=2) as ov_pool,
            tc.tile_pool(name="psum", bufs=7, space="PSUM") as ps_pool,
            tc.tile_pool(name="pso", bufs=1, space="PSUM") as pso_pool,
        ):
            # ---- resident constants (Wg before Wz: wgs GEMM comes first)
            u_sb = cpool.tile([128, 5 * H], mdt)
            nc.gpsimd.dma_start(u_sb[:], u_d[:])
            bias_sb = cpool.tile([128, 48], f32)
            nc.gpsimd.dma_start(bias_sb[:], bias_d[:])
            ow_sb = cpool.tile([128, NM], mdt)
            nc.gpsimd.dma_start(ow_sb[:], ow_d[:])
            wg_sb = cpool.tile([128, NM * H], mdt)
            wz_sb = cpool.tile([128, NM * H], mdt)
            for k in range(NM):
                nc.gpsimd.dma_start(wg_sb[:, k * H:(k + 1) * H],
                                    wg_d[k * 128:(k + 1) * 128, :])
            for k in range(NM):
                nc.gpsimd.dma_start(wz_sb[:, k * H:(k + 1) * H],
                                    wz_d[k * 128:(k + 1) * 128, :])

            def w_ap(w_sb, k, m):
                return w_sb[:, k * H + m * 128:k * H + (m + 1) * 128]

            def u_ap(g, m, c):
                return u_sb[32 * c:32 * c + KI,
                            g * H + m * 128:g * H + (m + 1) * 128]

            def b_ap(g, m):
                return bias_sb[:, g * NM + m:g * NM + m + 1]

            def x_project(gate, xt, dest, nametag, evac="act"):
                """8 K=16 matmuls (2 quads of 4 row-tiles), bias-folding
                evacuation on the chosen engine, into dest[m] (bf16 SBUF).
                Spreading evacs across ACT/DVE/Pool keeps the PE from being
                paced by a single evacuation engine (PSUM banks free up at
                evac rate)."""
                for mq in (0, 4):
                    pss = {}
                    for c in range(4):
                        m = mq + c
                        pss[m] = ps_pool.tile([128, nb], f32, tag="ps",
                                              name=f"{nametag}_{m}")
                    for c in range(4):
                        m = mq + c
                        nc.tensor.matmul(
                            pss[m][:], u_ap(gate, m, c),
                            xt[32 * c:32 * c + KI, :],
                            start=True, stop=True,
                            tile_position=(32 * c, 0))
                    for c in range(4):
                        m = mq + c
                        if evac == "act":
                            nc.scalar.activation(dest[m][:], pss[m][:],
                                                 Ident, bias=b_ap(gate, m))
                        else:
                            nc.vector.tensor_scalar_add(dest[m][:],
                                                        pss[m][:],
                                                        b_ap(gate, m))

            def gemm(w_sb, src, preload, dest, nametag, act=Tanh):
                """Per m: Pool-preload PSUM with preload[m] (or None), 8
                accumulating K=128 matmuls over src[k], ACT evac to
                dest[m]."""
                for mq in (0, 4):
                    pss = {}
                    for c in range(4):
                        m = mq + c
                        pss[m] = ps_pool.tile([128, nb], f32, tag="ps",
                                              name=f"{nametag}_{m}")
                        if preload is not None:
                            nc.vector.tensor_copy(pss[m][:], preload[m][:])
                    for c in range(4):
                        m = mq + c
                        first = preload is None
                        for k in range(NM):
                            nc.tensor.matmul(
                                pss[m][:], w_ap(w_sb, k, m), src[k][:],
                                start=(first and k == 0), stop=(k == NM - 1),
                                skip_group_check=not first)
                        nc.scalar.activation(dest[m][:], pss[m][:], act)

            def emit_out(pend):
                h_prev, tp, up = pend
                po = pso_pool.tile([1, nb], f32, tag="po", name=f"po_{up}")
                for k in range(NM):
                    nc.tensor.matmul(po[:], ow_sb[:, k:k + 1], h_prev[k][:],
                                     start=(k == 0), stop=(k == NM - 1))
                orow = ov_pool.tile([1, nb], f32, tag="orow",
                                    name=f"orow_{up}")
                nc.vector.tensor_scalar_add(orow[:], po[:],
                                            bias_sb[0:1, 40:41])
                nc.gpsimd.dma_start(y_d[0:1, tp * nb:(tp + 1) * nb], orow[:])

            pend = None

            for rep in range(repeat):
                for t in range(nt):
                    t_u = rep * nt + t
                    xt = xt_pool.tile([128, nb], mdt, tag="xt",
                                      name=f"xt_{t_u}")
                    for c in range(4):
                        nc.gpsimd.dma_start(xt[32 * c:32 * c + KI, :],
                                            xT_d[:, t * nb:(t + 1) * nb])

                    def mk(pool, tag_prefix, name):
                        return [pool.tile([128, nb], mdt, tag=f"{tag_prefix}{m}",
                                          name=f"{name}_{m}")
                                for m in range(NM)]

                    # ---- phase A: x projections, once per tile ----------
                    s_cur = mk(s_pool, "s", f"s_{t_u}_0")
                    x_project(0, xt, s_cur, f"psA_s1_{t_u}", evac="act")

                    # previous tile's scalar output row
                    if pend is not None:
                        emit_out(pend)
                        pend = None

                    ux_r = mk(ux_pool, "uxr", f"uxr_{t_u}")
                    x_project(3, xt, ux_r, f"psA_r_{t_u}", evac="dve")
                    ux_g = mk(ux_pool, "uxg", f"uxg_{t_u}")
                    x_project(2, xt, ux_g, f"psA_g_{t_u}", evac="act")
                    ux_z = mk(ux_pool, "uxz", f"uxz_{t_u}")
                    x_project(1, xt, ux_z, f"psA_z_{t_u}", evac="dve")
                    ux_h = mk(ux_pool, "uxh", f"uxh_{t_u}")
                    x_project(4, xt, ux_h, f"psA_h_{t_u}", evac="act")

                    # ---- phase B: wgs = Wg @ S1 (shared by G and R0) ----
                    # As each wgs[m] lands: G and R0 adds + tanh + S1*R0,
                    # overlapping the remaining wgs GEMM work on the PE.
                    wgs = [act_pool.tile([128, nb], mdt, tag=f"h{m}",
                                         name=f"wgs_{t_u}_{m}")
                           for m in range(NM)]
                    g_t = mk(act_pool, "g", f"g_{t_u}")
                    r_t = mk(act_pool, "r", f"r_{t_u}_0")
                    for mq in (0, 4):
                        pss = {}
                        for c in range(4):
                            m = mq + c
                            pss[m] = ps_pool.tile([128, nb], f32, tag="ps",
                                                  name=f"ps_wgs_{t_u}_{m}")
                        for c in range(4):
                            m = mq + c
                            for k in range(NM):
                                nc.tensor.matmul(pss[m][:], w_ap(wg_sb, k, m),
                                                 s_cur[k][:],
                                                 start=(k == 0),
                                                 stop=(k == NM - 1))
                            nc.scalar.activation(wgs[m][:], pss[m][:], Ident)
                        for c in range(4):
                            m = mq + c
                            nc.vector.tensor_add(g_t[m][:], wgs[m][:],
                                                 ux_g[m][:])
                            nc.scalar.activation(g_t[m][:], g_t[m][:], Tanh)
                            nc.vector.tensor_add(r_t[m][:], wgs[m][:],
                                                 ux_r[m][:])
                            nc.scalar.activation(r_t[m][:], r_t[m][:], Tanh)
                            nc.vector.tensor_mul(r_t[m][:], s_cur[m][:],
                                                 r_t[m][:])
                            # gm1 = G - 1, in place (combine uses Z*S-(G-1)*H)
                            nc.vector.tensor_scalar_sub(g_t[m][:], g_t[m][:],
                                                        1.0)

                    # ---- layers ----------------------------------------
                    h_t = None
                    for i in range(N_LAYERS):
                        if i > 0:
                            # R = tanh(ux_r + Wg S)
                            r_t = mk(act_pool, "r", f"r_{t_u}_{i}")
                            gemm(wg_sb, s_cur, ux_r, r_t,
                                 f"ps_r_{t_u}_{i}")
                        # Z = tanh(ux_z + Wz S)
                        z_t = mk(act_pool, "z", f"z_{t_u}_{i}")
                        gemm(wz_sb, s_cur, ux_z, z_t, f"ps_z_{t_u}_{i}")
                        if i > 0:
                            # SR = S * R (after R tanh; in place into r)
                            for m in range(NM):
                                nc.vector.tensor_mul(r_t[m][:], s_cur[m][:],
                                                     r_t[m][:])
                        # H = tanh(ux_h + Wg (S*R))
                        h_t = [act_pool.tile([128, nb], mdt, tag=f"h{m}",
                                             name=f"h_{t_u}_{i}_{m}")
                               for m in range(NM)]
                        gemm(wg_sb, r_t, ux_h, h_t, f"ps_h_{t_u}_{i}")

                        # output = Z*S - (G-1)*H ; then S_next = tanh(output)
                        if i < N_LAYERS - 1:
                            s_new = mk(s_pool, "s", f"s_{t_u}_{i + 1}")
                        for m in range(NM):
                            nc.vector.tensor_mul(z_t[m][:], z_t[m][:],
                                                 s_cur[m][:])
                            nc.vector.tensor_mul(h_t[m][:], g_t[m][:],
                                                 h_t[m][:])
                            nc.vector.tensor_sub(h_t[m][:], z_t[m][:],
                                                 h_t[m][:])
                            if i < N_LAYERS - 1:
                                nc.scalar.activation(s_new[m][:], h_t[m][:],
                                                     Tanh)
                        if i < N_LAYERS - 1:
                            s_cur = s_new

                    pend = (h_t, t, t_u)

            if pend is not None:
                emit_out(pend)

    nc.compile()
    return nc


def _get_nc(bc=BC, nb=NB, mm_dt=MM_DT):
    key = (bc, nb, mm_dt)
    if key not in _BUILD_CACHE:
        _BUILD_CACHE[key] = _build(bc, nb, mm_dt)
    return _BUILD_CACHE[key]


def _prep_inputs(x, Sw_w, Sw_b, Uz_w, Uz_b, Wz_w, Wz_b, Ug_w, Ug_b, Wg_w,
                 Wg_b, Ur_w, Ur_b, Uh_w, Uh_b, out_w, out_b):
    import ml_dtypes
    f = np.float32
    bf = ml_dtypes.bfloat16
    xT = np.ascontiguousarray(np.asarray(x, f).T).astype(bf)     # [16, B]
    WzT = np.ascontiguousarray(np.asarray(Wz_w, f).T).astype(bf)
    WgT = np.ascontiguousarray(np.asarray(Wg_w, f).T).astype(bf)
    U16 = np.concatenate(
        [np.asarray(w, f).T for w in (Sw_w, Uz_w, Ug_w, Ur_w, Uh_w)],
        axis=1)                                                  # [16, 5H]
    U = np.zeros((128, 5 * H), f)
    for c in range(4):
        U[32 * c:32 * c + KI] = U16
    U = U.astype(bf)
    bias = np.zeros((128, 48), f)
    combos = [
        np.asarray(Sw_b, f),
        np.asarray(Uz_b, f) + np.asarray(Wz_b, f),
        np.asarray(Ug_b, f) + np.asarray(Wg_b, f),
        np.asarray(Ur_b, f) + np.asarray(Wg_b, f),
        np.asarray(Uh_b, f) + np.asarray(Wg_b, f),
    ]
    for g, b in enumerate(combos):
        bias[:, g * NM:(g + 1) * NM] = b.reshape(NM, 128).T
    bias[:, 40] = np.float32(np.asarray(out_b, f)[0])
    OW = np.ascontiguousarray(
        np.asarray(out_w, f).reshape(NM, 128).T).astype(bf)
    return xT, WzT, WgT, U, bias, OW


def _make_in_maps(inputs):
    xT, WzT, WgT, U, bias, OW = _prep_inputs(**inputs)
    return [{
        "xT": np.ascontiguousarray(xT[:, c * BC:(c + 1) * BC]),
        "WzT": WzT, "WgT": WgT, "U": U, "BIAS": bias, "OW": OW,
    } for c in range(NCORES)]


def kernel(**inputs):
    from concourse.bass_utils import run_bass_kernel_spmd

    nc = _get_nc()
    in_maps = _make_in_maps(inputs)
    res = run_bass_kernel_spmd(nc, in_maps, list(range(NCORES)))
    y = np.concatenate([res.results[c]["Y"] for c in range(NCORES)], axis=1)
    return np.ascontiguousarray(y.reshape(B_FULL, 1)).astype(np.float32)


def timed_run(inputs, iters=5, nc=None, pipeline=1):
    import hwtime
    if nc is None:
        nc = _get_nc()
    in_maps = _make_in_maps(inputs)
    run_once, get_out = hwtime.make_runner(nc, in_maps, NCORES,
                                           pipeline=pipeline)

    def get_y():
        y = get_out("Y")  # [8, BC]
        return np.ascontiguousarray(
            y.reshape(1, B_FULL).reshape(B_FULL, 1)).astype(np.float32)

    if iters is None:
        return run_once, get_y
    import numpy as _np
    times = [run_once() for _ in range(iters)]
    return min(times), times, get_y()
